# revision 10
# baseline (speedup 1.0000x reference)
"""Trainium2 Bass kernel for nn_Attention — zero-collective nq sharding.

Core c = (b, j) with b = c//4, j = c%4. Each core computes the FULL
attention output rows for nq block j (512 tokens) of batch b: it builds
K^T/V for ALL 16 heads over the full sequence (replicated across the 4
nq cores of a batch — ~4.3 GFLOP of redundant QKV work), runs attention
for its 512 queries, and projects locally. NO collectives: cores are
fully independent, so the measured span never includes cross-core launch
skew (which dominated the collective variant: the first-collective
barrier absorbs up to ~100-250us of PJRT dispatch skew).

Attention engine split (per head-group hg of 4 heads, per nk tile t):
QKT is a 4-head row-banded pack (K=32 each); exp splits across ScalarE
(heads 0-1 exact, [128,1024] one instr) and VectorE (heads 2-3 via a
one-pass Schraudolph: int16(st*A + B) bits == bf16(exp(SCALE*st)), two
[128,512] instrs so stB tiles are single-bank and double-buffer in 2
PSUM banks). AV + ones-denominator packs are col-banded K=128 matmuls.
Host passes all tensors pre-relaid so every DMA line is contiguous.
"""

import os

import numpy as np
import ml_dtypes

import concourse.bass as bass
import concourse.bacc as bacc
import concourse.mybir as mybir
import concourse.tile as tile
from concourse.bass_utils import run_bass_kernel_spmd

B, N, D = 2, 2048, 1024
H, HD, CD = 16, 32, 512            # heads, cur head dim, cur dim
NCORES = 8
SCALE = (64 ** -0.5) / (0.5 ** 0.5)
BF = mybir.dt.bfloat16
F32 = mybir.dt.float32
I16 = mybir.dt.int16
AF = mybir.ActivationFunctionType
ALU = mybir.AluOpType

NQB = 512                          # per-core nq block (one PSUM bank fp32)
NKT = N // 128                     # 16 nk tiles
DT = D // 128                      # 8 contraction tiles over model dim

LOG2E = 1.4426950408889634
EXP_A = SCALE * LOG2E * 128.0
EXP_B = 16251.0 + 0.5
ACT_EVERY = 4                      # every 4th t, ScalarE takes one stB half


def build_nc():
    nc = bacc.Bacc(num_devices=NCORES)

    xre = nc.dram_tensor("xre", [128, DT * N], BF, kind="ExternalInput")
    xq = nc.dram_tensor("xq", [128, DT * NQB], BF, kind="ExternalInput")
    wk = nc.dram_tensor("wk", [128, DT * CD], BF, kind="ExternalInput")
    wq = nc.dram_tensor("wq", [128, DT * CD], BF, kind="ExternalInput")
    wv = nc.dram_tensor("wv", [128, DT * CD], BF, kind="ExternalInput")
    wp = nc.dram_tensor("wp", [128, 4 * D], BF, kind="ExternalInput")
    biasT = nc.dram_tensor("biasT", [128, 8], F32, kind="ExternalInput")
    onesb = nc.dram_tensor("onesb", [128, 32], BF, kind="ExternalInput")
    out = nc.dram_tensor("out", [D, NQB], F32, kind="ExternalOutput")

    with tile.TileContext(nc) as tc:
        with (
            tc.tile_pool(name="wpp", bufs=1) as wpp,
            tc.tile_pool(name="ptp", bufs=3) as ptp,
            tc.tile_pool(name="finp", bufs=2) as finp,
        ):
            # ---- input DMAs (all contiguous lines, host pre-relaid) ----
            # interleave wk/x per-dt chunks so the first K^T matmuls can
            # issue ~5us in instead of waiting for whole-tensor DMAs
            wk_sb = wpp.tile([128, DT, CD], BF)
            x_sb = wpp.tile([128, DT, N], BF)
            for dt in range(DT):
                nc.sync.dma_start(
                    wk_sb[:, dt, :], wk[:, CD * dt:CD * (dt + 1)])
                nc.sync.dma_start(
                    x_sb[:, dt, :], xre[:, N * dt:N * (dt + 1)])
            wv_sb = wpp.tile([128, DT, CD], BF)
            nc.sync.dma_start(wv_sb[:], wv[:].rearrange("p (a n) -> p a n", a=DT))
            xq_sb = wpp.tile([128, DT, NQB], BF)
            nc.sync.dma_start(xq_sb[:], xq[:].rearrange("p (a n) -> p a n", a=DT))
            wq_sb = wpp.tile([128, DT, CD], BF)
            nc.sync.dma_start(wq_sb[:], wq[:].rearrange("p (a n) -> p a n", a=DT))
            ones_sb = wpp.tile([128, 32], BF)
            nc.scalar.dma_start(ones_sb[:], onesb[:])
            wpj_sb = wpp.tile([128, 4, D], BF)
            nc.gpsimd.dma_start(
                wpj_sb[:], wp[:].rearrange("p (a n) -> p a n", a=4))
            bias_sb = wpp.tile([128, 8], F32)
            nc.gpsimd.dma_start(bias_sb[:], biasT[:])

            kt_sb = wpp.tile([128, 4, N], BF)      # [kdim-block hg][nk]
            qt_sb = wpp.tile([128, 4, NQB], BF)    # [qdim-block hg][nq]
            v_sb = wpp.tile([128, NKT, CD], BF)    # [nk-part][t][vdim]
            ot_sb = wpp.tile([128, 4, NQB], BF)    # [odim-block hg][nq]
            recs_sb = wpp.tile([128, NQB], F32)
            warm_sb = wpp.tile([128, 1], BF)

            nc.scalar.activation(warm_sb[:], ones_sb[:, 0:1], AF.Exp)

            copy_flip = [0]

            def emit_copy(dst, src):
                # alternate psum->sbuf copies between ACT and DVE
                if copy_flip[0] % 2 == 0:
                    nc.scalar.copy(dst, src)
                else:
                    nc.vector.tensor_copy(dst, src)
                copy_flip[0] += 1

            # ---- phase 1: K^T (hg-major), Q^T, V (n-major) ----
            with tc.tile_pool(name="ps1", bufs=8,
                              space=bass.MemorySpace.PSUM) as ps1:
                # wake the PE clock while the x DMA streams in
                wrm = ps1.tile([128, NQB], F32, tag="s1", name="wrm")
                for w in range(16):
                    nc.tensor.matmul(
                        wrm[:], wk_sb[:, 0, 128 * (w % 4):128 * (w % 4 + 1)],
                        wk_sb[:, 0, :],
                        start=True, stop=True,
                    )
                # K^T in two passes of 8 live accumulators, dt-outer so
                # matmuls chase the x chunks as they land.
                for half in range(2):
                    accs = [
                        ps1.tile([128, NQB], F32, tag="s1", name=f"k{half}{i}")
                        for i in range(8)
                    ]
                    for dt in range(DT):
                        for i in range(8):
                            rb, nb = 2 * half + i // 4, i % 4
                            nc.tensor.matmul(
                                accs[i][:],
                                wk_sb[:, dt, 128 * rb:128 * (rb + 1)],
                                x_sb[:, dt, NQB * nb:NQB * (nb + 1)],
                                start=(dt == 0), stop=(dt == DT - 1),
                            )
                    for i in range(8):
                        rb, nb = 2 * half + i // 4, i % 4
                        emit_copy(
                            kt_sb[:, rb, NQB * nb:NQB * (nb + 1)], accs[i][:])
                # V, n-major: one [128(nk), 512(vdim)] tile per t
                for g in range(4):
                    vaccs = [
                        ps1.tile([128, NQB], F32, tag="s1", name=f"v{g}{i}")
                        for i in range(4)
                    ]
                    for dt in range(DT):
                        for i in range(4):
                            t = 4 * g + i
                            nc.tensor.matmul(
                                vaccs[i][:],
                                x_sb[:, dt, 128 * t:128 * (t + 1)],
                                wv_sb[:, dt, :],
                                start=(dt == 0), stop=(dt == DT - 1),
                            )
                    for i in range(4):
                        emit_copy(v_sb[:, 4 * g + i, :], vaccs[i][:])
                # Q^T for my nq block only
                qaccs = [
                    ps1.tile([128, NQB], F32, tag="s1", name=f"q{i}")
                    for i in range(4)
                ]
                for dt in range(DT):
                    for qb in range(4):
                        nc.tensor.matmul(
                            qaccs[qb][:],
                            wq_sb[:, dt, 128 * qb:128 * (qb + 1)],
                            xq_sb[:, dt, :],
                            start=(dt == 0), stop=(dt == DT - 1),
                        )
                for qb in range(4):
                    emit_copy(qt_sb[:, qb, :], qaccs[qb][:])

            # ---- phase 2: attention (64 iters of (hg, t)), then proj ----
            with (
                tc.tile_pool(name="sta", bufs=2, space=bass.MemorySpace.PSUM) as sta,
                tc.tile_pool(name="stb", bufs=2, space=bass.MemorySpace.PSUM) as stb,
                tc.tile_pool(name="ov", bufs=1, space=bass.MemorySpace.PSUM) as ovp,
                tc.tile_pool(name="sm", bufs=1, space=bass.MemorySpace.PSUM) as smp,
            ):
                o_accs, s_accs = {}, {}

                def emit_qkt_exp(hg, t):
                    # heads 2,3 (DVE leg) first in single-bank tiles; heads
                    # 0,1 (ACT leg) in a contiguous 2-bank tile so the exp
                    # is one big ScalarE instruction.
                    stB0 = stb.tile([128, NQB], F32, tag="stB", name="stB0")
                    stB1 = stb.tile([128, NQB], F32, tag="stB", name="stB1")
                    stA = sta.tile([128, 2 * NQB], F32, tag="stA", name="stA")
                    for h, dst, co in (
                        (2, stB0, 0), (3, stB1, 0),
                        (0, stA, 0), (1, stA, NQB),
                    ):
                        tp = (32 * h, 0) if h == 3 else None
                        nc.tensor.matmul(
                            dst[:, co:co + NQB],
                            kt_sb[32 * h:32 * (h + 1), hg, 128 * t:128 * (t + 1)],
                            qt_sb[32 * h:32 * (h + 1), hg, :],
                            start=True, stop=True, tile_position=tp,
                        )
                    ptA = ptp.tile([128, 2 * NQB], I16, tag="pA")
                    ptB = ptp.tile([128, 2 * NQB], I16, tag="pB")
                    if t % ACT_EVERY == ACT_EVERY - 1:
                        # rebalance: ACT takes one stB half this iter
                        nc.scalar.activation(
                            ptB[:, 0:NQB].bitcast(BF), stB0[:], AF.Exp,
                            scale=SCALE)
                    else:
                        nc.vector.tensor_scalar(
                            ptB[:, 0:NQB], stB0[:], EXP_A, EXP_B,
                            ALU.mult, ALU.add)
                    nc.vector.tensor_scalar(
                        ptB[:, NQB:], stB1[:], EXP_A, EXP_B,
                        ALU.mult, ALU.add)
                    nc.scalar.activation(
                        ptA[:].bitcast(BF), stA[:], AF.Exp, scale=SCALE)
                    return ptA, ptB

                def emit_av(hg, t, ptA, ptB):
                    o_acc, s_acc = o_accs[hg], s_accs[hg]
                    for h in range(4):
                        pt = ptA if h < 2 else ptB
                        rhs = pt[:, NQB * (h % 2):NQB * (h % 2 + 1)].bitcast(BF)
                        nc.tensor.matmul(
                            o_acc[32 * h:32 * (h + 1), :],
                            v_sb[:, t, 128 * hg + 32 * h:128 * hg + 32 * (h + 1)],
                            rhs,
                            start=(t == 0), stop=(t == NKT - 1),
                            tile_position=(0, 32 * h),
                        )
                    for h in range(4):
                        pt = ptA if h < 2 else ptB
                        rhs = pt[:, NQB * (h % 2):NQB * (h % 2 + 1)].bitcast(BF)
                        nc.tensor.matmul(
                            s_acc[32 * h:32 * (h + 1), :],
                            ones_sb[:],
                            rhs,
                            start=(t == 0), stop=(t == NKT - 1),
                            tile_position=(0, 32 * h),
                        )

                def emit_norm(hg):
                    nc.vector.reciprocal_approx_fast(recs_sb[:], s_accs[hg][:])
                    nc.vector.tensor_tensor(
                        ot_sb[:, hg, :], o_accs[hg][:], recs_sb[:], ALU.mult)

                def alloc_accs(hg):
                    o_accs[hg] = ovp.tile([128, NQB], F32, tag="o", name=f"o{hg}")
                    s_accs[hg] = smp.tile([128, NQB], F32, tag="sm", name=f"s{hg}")

                sched = [(hg, t) for hg in range(4) for t in range(NKT)]
                pts = {0: emit_qkt_exp(*sched[0])}
                alloc_accs(0)
                for i, (hg, t) in enumerate(sched):
                    if i + 1 < len(sched):
                        pts[i + 1] = emit_qkt_exp(*sched[i + 1])
                    emit_av(hg, t, *pts.pop(i))
                    if t == NKT - 1:
                        emit_norm(hg)
                        if hg < 3:
                            alloc_accs(hg + 1)

                # ---- proj: out.T[128rb:, my nq] = sum_hg Wp-block @ ot ----
                for rb in range(8):
                    acc = stb.tile([128, NQB], F32, tag="stB", name=f"pj{rb}")
                    for hg in range(4):
                        nc.tensor.matmul(
                            acc[:],
                            wpj_sb[:, hg, 128 * rb:128 * (rb + 1)],
                            ot_sb[:, hg, :],
                            start=(hg == 0), stop=(hg == 3),
                        )
                    fin = finp.tile([128, NQB], F32, tag="fin")
                    nc.vector.tensor_scalar(
                        fin[:], acc[:], bias_sb[:, rb:rb + 1], None, ALU.add)
                    nc.sync.dma_start(out[128 * rb:128 * (rb + 1), :], fin[:])
    nc.compile()
    return nc


_NC = None


def _relay(a, nblk):
    # [nblk*128, m] -> [128, nblk*m] with line p = concat over blocks
    m = a.shape[1]
    return np.ascontiguousarray(
        a.reshape(nblk, 128, m).transpose(1, 0, 2).reshape(128, nblk * m))


def kernel(x, w_qkv, w_proj, b_proj):
    global _NC
    if _NC is None:
        _NC = build_nc()
    bf = ml_dtypes.bfloat16

    wqkvT = np.ascontiguousarray(w_qkv[:3 * CD].T)             # [D, 1536]
    wq_h = _relay(wqkvT[:, 0:CD], DT).astype(bf)
    wk_h = _relay(wqkvT[:, CD:2 * CD], DT).astype(bf)
    wv_h = _relay(wqkvT[:, 2 * CD:3 * CD], DT).astype(bf)
    wp_h = _relay(np.ascontiguousarray(w_proj[:, :CD].T), 4).astype(bf)
    bias_h = np.ascontiguousarray(
        b_proj.astype(np.float32).reshape(8, 128).T)
    onesb = np.ones((128, 32), dtype=bf)

    xre_b, xq_b = [], []
    for b in range(B):
        xT = np.ascontiguousarray(x[b].T)                      # [D, N]
        xr3 = xT.reshape(DT, 128, N).transpose(1, 0, 2)        # [128, DT, N]
        xre_b.append(np.ascontiguousarray(
            xr3.reshape(128, DT * N)).astype(bf))
        xq_b.append([
            np.ascontiguousarray(
                xr3[:, :, NQB * j:NQB * (j + 1)].reshape(128, DT * NQB)
            ).astype(bf)
            for j in range(4)
        ])

    in_maps = []
    for c in range(NCORES):
        b, j = c // 4, c % 4
        in_maps.append({
            "xre": xre_b[b],
            "xq": xq_b[b][j],
            "wk": wk_h, "wq": wq_h, "wv": wv_h, "wp": wp_h,
            "biasT": bias_h,
            "onesb": onesb,
        })

    trace = bool(os.environ.get("KERNEL_TRACE"))
    rr = run_bass_kernel_spmd(
        _NC, in_maps, list(range(NCORES)),
        trace=trace, tmpdir=os.environ.get("KERNEL_TRACE_DIR") or None,
    )
    if rr.exec_time_ns is not None:
        print(f"HW exec time: {rr.exec_time_ns} ns")
    res = rr.results

    outp = np.empty((B, N, D), dtype=np.float32)
    for c in range(NCORES):
        b, j = c // 4, c % 4
        outp[b, NQB * j:NQB * (j + 1), :] = res[c]["out"].T
    return outp
